# revision 30
# baseline (speedup 1.0000x reference)
"""Multi-head self-attention (B=4, T=2048, C=768, H=12) on 8 trn2 NeuronCores.

Sharding: core c -> batch b=c//2, head-group g=c%2 (6 heads each).
Each core computes its 6 heads' attention and a partial output projection
(contraction over its 384 ctx dims). Host sums the 2 partials per batch
and adds the bias.

Per-core kernel, software-pipelined to keep the PE saturated (HAM at 8/8):
  - batched DMAs: X in 4 gathers of 512 rows, each weight in one gather.
  - X^T via PE transposes (xps pool); weights used as f32r directly.
  - qT/kT [384,2048] bf16 projections (w^T x^T form); the softmax scale
    1/8 is folded into the qT copy so the ACT exp needs no scale.
  - v computed directly in [T, 384] orientation (lhsT = x^T chunk), no
    v transposes; ones column appended -> va[128, t, 6, 65] fp8 gives the
    softmax denominators out of the ctx matmul for free.
  - attention stream over (half, head, j): scores^T chunk (bf16) -> exp on
    ACT -> ctx^T accumulated with fp8 DoubleRow over j-PAIRS (the exp of
    j=2p,2p+1 lands in one [128,2048] fp8 tile = the DoubleRow rhs), ctx
    lagging scores by a pair so the PE never blocks on the ACT. All other
    projection work, X-transposes, and the output projection are queued as
    "extras" drained into the PE stream (forced prerequisite drains +
    token bucket), so the PE stays busy end-to-end.
  - normalize: R = ones x recip(sums) (PE outer product), ctxT = cu * R.
  - out[t] = sum_m ctxT[m].T @ Wo[m] -> DMA out (partial, pre-bias).

PSUM budget (8 banks): sps 2x[128,1024] (4) + cps [65,1024] (2) +
xps 2x[128,512] (2, shared by transposes/proj/v/rps/outproj).
"""
import sys
import os

sys.path.insert(0, "/opt/trn_rl_repo")

import numpy as np

P = 128
T = 2048
C = 768
HD = 384          # per-core head columns (6 heads x 64)
D = 64
NT = T // P       # 16 T chunks of 128
KC = C // P       # 6 contraction chunks for C
MC = HD // P      # 3 chunks of head dims
NH = 6            # heads per core
HALF = 1024       # T_q blocking for the attention inner loop
NN = T // 512     # 4 proj n-chunks of 512

_cache = {}


def _build():
    import concourse.bacc as bacc
    import concourse.mybir as mybir
    import concourse.tile as tile
    from concourse.masks import make_identity
    from contextlib import ExitStack

    F32 = mybir.dt.float32
    F32R = mybir.dt.float32r
    BF16 = mybir.dt.bfloat16
    FP8 = mybir.dt.float8e4
    AF = mybir.ActivationFunctionType
    ALU = mybir.AluOpType
    DR = mybir.MatmulPerfMode.DoubleRow

    nc = bacc.Bacc("TRN2", target_bir_lowering=False, debug=False)
    x = nc.dram_tensor("x", [T, C], BF16, kind="ExternalInput").ap()
    wq = nc.dram_tensor("wq", [C, HD], BF16, kind="ExternalInput").ap()
    wk = nc.dram_tensor("wk", [C, HD], BF16, kind="ExternalInput").ap()
    wv = nc.dram_tensor("wv", [C, HD], BF16, kind="ExternalInput").ap()
    wo = nc.dram_tensor("wo", [HD, C], F32R, kind="ExternalInput").ap()
    out = nc.dram_tensor("out", [T, C], F32, kind="ExternalOutput").ap()

    def emit(tc, ctx):
        consts = ctx.enter_context(tc.tile_pool(name="consts", bufs=1))
        ident = consts.tile([P, P], F32)
        make_identity(nc, ident)
        ident_bf = consts.tile([P, P], BF16)
        nc.vector.tensor_copy(ident_bf[:], ident[:])
        ones_r = consts.tile([1, D], F32R)
        nc.vector.tensor_scalar(ones_r[:], ident[0:1, 0:D], 0.0, 1.0, ALU.mult, ALU.add)

        big = ctx.enter_context(tc.tile_pool(name="big", bufs=1))
        xt = [big.tile([P, T], BF16, name=f"xt{kc}", tag=f"xt{kc}") for kc in range(KC)]
        qT = [big.tile([P, T], BF16, name=f"qT{m}", tag=f"qT{m}") for m in range(MC)]
        kT = [big.tile([P, T], BF16, name=f"kT{m}", tag=f"kT{m}") for m in range(MC)]
        ctxT = [big.tile([P, T], F32R, name=f"cT{m}", tag=f"cT{m}") for m in range(MC)]
        # va layout [p, h, t, 80]: cols 0..64 used (64 v dims + ones), rest pad
        VZ = 80
        va = big.tile([P, NH * NT * VZ], BF16, name="va", tag="va")
        va4 = va[:].rearrange("p (h t z) -> p h t z", h=NH, t=NT)
        # weight slabs: one DMA each; w[nm][:, 384*kc+...] is chunk kc
        wsl = {}
        for nm in ("q", "k", "v"):
            wsl[nm] = big.tile([P, KC * HD], BF16, name=f"w{nm}", tag=f"w{nm}")
        wosl = big.tile([P, MC * C], F32R, name="wo", tag="wo")
        w_r = {(nm, kc): wsl[nm][:, HD * kc:HD * (kc + 1)]
               for nm in ("q", "k", "v") for kc in range(KC)}
        wo_r = [wosl[:, C * m:C * (m + 1)] for m in range(MC)]

        xrp = ctx.enter_context(tc.tile_pool(name="xrp", bufs=2))
        ptp = ctx.enter_context(tc.tile_pool(name="ptp", bufs=3))
        outp = ctx.enter_context(tc.tile_pool(name="outp", bufs=2))
        norm = ctx.enter_context(tc.tile_pool(name="norm", bufs=1))
        xps = ctx.enter_context(tc.tile_pool(name="xps", bufs=2, space="PSUM"))
        sps = ctx.enter_context(tc.tile_pool(name="sps", bufs=2, space="PSUM"))
        cps = ctx.enter_context(tc.tile_pool(name="cps", bufs=1, space="PSUM"))

        # denominator ones column, before anything reads va
        nc.vector.tensor_scalar(
            va4[:, :, :, D:D + 1],
            ident[:, 0:NH * NT].rearrange("p (h t c) -> p h t c", h=NH, t=NT),
            0.0, 1.0, ALU.mult, ALU.add)

        # ---- batched input DMAs (X group g = rows [512g, 512(g+1)))
        xg = []
        def x_dma(g):
            xr = xrp.tile([P, 4 * C], BF16, name=f"xg{g}", tag="xr")
            nc.sync.dma_start(
                xr[:].rearrange("p (i c) -> p i c", i=4),
                x[512 * g:512 * (g + 1), :].rearrange("(i p) c -> p i c", p=P))
            xg.append(xr)
        x_dma(0)
        x_dma(1)
        for nm, src in (("k", wk), ("q", wq), ("v", wv)):
            nc.sync.dma_start(
                wsl[nm][:].rearrange("p (i c) -> p i c", i=KC),
                src[:].rearrange("(i p) c -> p i c", p=P))
        x_dma(2)
        x_dma(3)
        nc.sync.dma_start(
            wosl[:].rearrange("p (i c) -> p i c", i=MC),
            wo[:].rearrange("(i p) c -> p i c", p=P))

        # PE warm-up while the first DMAs land: dependency-free matmuls ramp
        # the HAM clock to 2.4 GHz before the real pipeline starts.
        for i in range(50):
            dps = xps.tile([P, 512], F32, name=f"warm{i}", tag="xps")
            nc.tensor.matmul(dps[:, 0:P], ident_bf[:], ident_bf[:],
                             start=True, stop=True)

        done = set()

        def xt_group(tq):
            """transpose X rows [512tq, 512(tq+1)) into xt[kc][:, 512tq:...]"""
            for kc in range(KC):
                # bf16 [128,512] = 1KB fits the 2KB "xps" slot; sharing the
                # tag keeps transposes off the sps ring (no exp-reader waits)
                tp = xps.tile([P, 512], BF16, name=f"tp{tq}_{kc}", tag="xps")
                for i in range(4):
                    nc.tensor.transpose(tp[:, P * i:P * (i + 1)],
                                        xg[tq][:, C * i + P * kc:C * i + P * (kc + 1)],
                                        ident_bf[:])
                nc.vector.tensor_copy(xt[kc][:, 512 * tq:512 * (tq + 1)], tp[:])
            done.add(("xt", tq))

        def qk_chunk(nm, m, n):
            """qT/kT[m][:, 512n:512(n+1)] (bf16); q gets the 1/8 scale"""
            dest = qT if nm == "q" else kT
            ps = xps.tile([P, 512], F32, name=f"ps{nm}{m}{n}", tag="xps")
            for kc in range(KC):
                nc.tensor.matmul(ps[:], w_r[nm, kc][:, P * m:P * (m + 1)],
                                 xt[kc][:, 512 * n:512 * (n + 1)],
                                 start=(kc == 0), stop=(kc == KC - 1))
            if nm == "q":
                nc.vector.tensor_scalar(dest[m][:, 512 * n:512 * (n + 1)], ps[:],
                                        float(D) ** -0.5, 0.0, ALU.mult, ALU.add)
            else:
                nc.vector.tensor_copy(dest[m][:, 512 * n:512 * (n + 1)], ps[:])
            done.add((f"qk{m}", nm, n))

        def v_chunk(t_i):
            """va[:, t_i, :, 0:64] = (X @ Wv) rows [128t_i, 128(t_i+1))  (fp8)"""
            ps = xps.tile([P, 512], F32, name=f"pv{t_i}", tag="xps")
            for kc in range(KC):
                nc.tensor.matmul(ps[:, 0:HD], xt[kc][:, P * t_i:P * (t_i + 1)],
                                 w_r["v", kc][:], start=(kc == 0), stop=(kc == KC - 1))
            nc.vector.tensor_copy(va4[:, :, t_i, 0:D],
                                  ps[:, 0:HD].rearrange("p (h c) -> p h c", h=NH))
            done.add(("v", t_i))

        def outproj(t_i):
            psA = xps.tile([P, 512], F32, name=f"psA{t_i}", tag="xps")
            psB = xps.tile([P, 512], F32, name=f"psB{t_i}", tag="xps")
            for m in range(MC):
                nc.tensor.matmul(psA[:], ctxT[m][:, P * t_i:P * (t_i + 1)],
                                 wo_r[m][:, 0:512], start=(m == 0), stop=(m == MC - 1))
                nc.tensor.matmul(psB[:, 0:C - 512], ctxT[m][:, P * t_i:P * (t_i + 1)],
                                 wo_r[m][:, 512:C], start=(m == 0), stop=(m == MC - 1))
            ob = outp.tile([P, C], F32, name=f"ob{t_i}", tag="ob")
            if t_i >= NT // 2:
                # tail chunks copy on the (idle) ACT engine so the DVE can
                # run the last normalize in parallel
                nc.scalar.copy(ob[:, 0:512], psA[:])
                nc.scalar.copy(ob[:, 512:C], psB[:, 0:C - 512])
            else:
                nc.vector.tensor_copy(ob[:, 0:512], psA[:])
                nc.vector.tensor_copy(ob[:, 512:C], psB[:, 0:C - 512])
            nc.sync.dma_start(out[P * t_i:P * (t_i + 1), :], ob[:])
            done.add(("po", t_i))

        # ---- extras queue: (label, cost_ns, thunk); queue order respects deps
        extras = []
        for t_i in range(2):
            extras.append((("v", t_i), 1300, lambda t_i=t_i: v_chunk(t_i)))
        extras.append((("qk0", "k", 1), 1400, lambda: qk_chunk("k", 0, 1)))
        extras.append((("xt", 2), 1100, lambda: xt_group(2)))
        extras.append((("xt", 3), 1100, lambda: xt_group(3)))
        for n in (2, 3):
            extras.append((("qk0", "k", n), 1400, lambda n=n: qk_chunk("k", 0, n)))
        for t_i in range(2, NT):
            extras.append((("v", t_i), 1300, lambda t_i=t_i: v_chunk(t_i)))
        for n in (2, 3):
            extras.append((("qk0", "q", n), 1400, lambda n=n: qk_chunk("q", 0, n)))
        for m in (1, 2):
            for nm, n in (("k", 0), ("k", 1), ("q", 0), ("q", 1), ("k", 2), ("k", 3)):
                extras.append(((f"qk{m}", nm, n), 1400,
                               lambda nm=nm, m=m, n=n: qk_chunk(nm, m, n)))
        for m in (1, 2):
            for n in (2, 3):
                extras.append(((f"qk{m}", "q", n), 1400,
                               lambda m=m, n=n: qk_chunk("q", m, n)))

        def drain_one():
            label, cost, th = extras.pop(0)
            th()
            return cost

        def drain_until(pred):
            while extras and not pred():
                drain_one()

        def need(*labels):
            req = set(labels)
            drain_until(lambda: req <= done)

        # ---- startup: enough for h0's first scores, emitted inline
        xt_group(0)
        qk_chunk("k", 0, 0)
        qk_chunk("q", 0, 0)
        xt_group(1)
        qk_chunk("q", 0, 1)

        # ---- the attention stream
        units = [(half, h, j) for half in (0, 1) for h in range(NH)
                 for j in range(NT)]
        cps_of = {}
        pt_of = {}
        pending = None     # (half, h, j) ctx waiting for emission
        bucket = 0.0

        def emit_scores_exp(u):
            half, h, j = u
            m, par = divmod(h, 2)
            need((f"qk{m}", "q", 2 * half), (f"qk{m}", "q", 2 * half + 1),
                 (f"qk{m}", "k", j // 4))
            qh = qT[m][D * par:D * (par + 1), HALF * half:HALF * (half + 1)]
            kh = kT[m][D * par:D * (par + 1), :]
            sp = sps.tile([P, HALF], F32, name=f"s{half}{h}{j}", tag="sps")
            for u2 in range(2):
                nc.tensor.matmul(sp[:, 512 * u2:512 * (u2 + 1)],
                                 kh[:, P * j:P * (j + 1)],
                                 qh[:, 512 * u2:512 * (u2 + 1)],
                                 start=True, stop=True)
            pt = ptp.tile([P, HALF], BF16, name=f"pt{half}{h}{j}", tag="pt")
            nc.scalar.activation(pt[:], sp[:], AF.Exp)
            pt_of[u] = pt

        def emit_ctx(pu):
            half, h, j = pu
            need(("v", j))
            if j == 0:
                cps_of[half, h] = cps.tile([D + 1, HALF], F32,
                                           name=f"c{half}{h}", tag="cps")
            cp = cps_of[half, h]
            pt = pt_of.pop(pu)
            for u2 in range(2):
                nc.tensor.matmul(cp[:, 512 * u2:512 * (u2 + 1)],
                                 va4[:, h, j, 0:D + 1],
                                 pt[:, 512 * u2:512 * (u2 + 1)],
                                 start=(j == 0), stop=(j == NT - 1))
            if j == NT - 1:
                normalize(half, h)

        def normalize(half, h):
            m, par = divmod(h, 2)
            cp = cps_of.pop((half, h))
            q0 = HALF * half
            cu = norm.tile([D, HALF], F32, name=f"cu{half}{h}", tag="cu")
            nc.vector.tensor_copy(cu[:], cp[0:D, :])
            # sums row staged on ACT in parallel with the cu copy, so the
            # cps bank frees fast and the next head's ctx doesn't stall
            s_sb = norm.tile([1, HALF], F32, name=f"sb{half}{h}", tag="ssb")
            nc.scalar.copy(s_sb[:], cp[D:D + 1, :])
            rr = norm.tile([1, HALF], F32, name=f"rr{half}{h}", tag="rr")
            nc.vector.reciprocal_approx_fast(rr[:], s_sb[:])
            rr_r = norm.tile([1, HALF], F32R, name=f"rc{half}{h}", tag="rrr")
            nc.vector.tensor_copy(rr_r[:], rr[:])
            for u2 in range(2):
                rp = xps.tile([P, 512], F32, name=f"rp{half}{h}{u2}", tag="xps")
                nc.tensor.matmul(rp[0:D, :], ones_r[:],
                                 rr_r[:][:, 512 * u2:512 * (u2 + 1)],
                                 start=True, stop=True)
                nc.vector.tensor_mul(
                    ctxT[m][D * par:D * (par + 1), q0 + 512 * u2:q0 + 512 * (u2 + 1)],
                    cu[:, 512 * u2:512 * (u2 + 1)], rp[0:D, :])
            if h == NH - 1:
                t_lo = 0 if half == 0 else NT // 2
                for t_i in range(t_lo, t_lo + NT // 2):
                    extras.append((("po", t_i), 1100, lambda t_i=t_i: outproj(t_i)))

        for u in units:
            emit_scores_exp(u)
            if pending is not None:
                emit_ctx(pending)
            pending = u
            bucket = min(bucket + 390.0, 4200.0)
            while extras and bucket >= extras[0][1]:
                bucket -= drain_one()
        emit_ctx(pending)
        while extras:
            drain_one()

    with tile.TileContext(nc) as tc, ExitStack() as ctx:
        emit(tc, ctx)

    nc.compile()
    return nc


def kernel(X, Wq, Wk, Wv, Wo, bo):
    from concourse import bass_utils

    if "nc" not in _cache:
        _cache["nc"] = _build()
    nc = _cache["nc"]

    from ml_dtypes import bfloat16

    X = np.asarray(X, dtype=np.float32).astype(bfloat16)
    Wq = np.asarray(Wq, np.float32).astype(bfloat16)
    Wk = np.asarray(Wk, np.float32).astype(bfloat16)
    Wv = np.asarray(Wv, np.float32).astype(bfloat16)
    in_maps = []
    for c in range(8):
        b, g = divmod(c, 2)
        sl = slice(HD * g, HD * (g + 1))
        in_maps.append({
            "x": np.ascontiguousarray(X[b]),
            "wq": np.ascontiguousarray(Wq[:, sl]),
            "wk": np.ascontiguousarray(Wk[:, sl]),
            "wv": np.ascontiguousarray(Wv[:, sl]),
            "wo": np.ascontiguousarray(np.asarray(Wo, np.float32)[sl, :]),
        })
    res = bass_utils.run_bass_kernel_spmd(nc, in_maps, core_ids=list(range(8)))
    _cache["last_res"] = res
    outf = np.empty((4, T, C), np.float32)
    bo = np.asarray(bo, np.float32)
    for b in range(4):
        outf[b] = res.results[2 * b]["out"] + res.results[2 * b + 1]["out"] + bo
    return outf


# revision 33
# speedup vs baseline: 1.0970x; 1.0970x over previous
"""Multi-head self-attention (B=4, T=2048, C=768, H=12) on 8 trn2 NeuronCores.

Sharding: core c -> batch b=c//2, head-group g=c%2 (6 heads each).
Each core computes its 6 heads' attention and a partial output projection
(contraction over its 384 ctx dims). Host sums the 2 partials per batch
and adds the bias.

Per-core kernel, software-pipelined to keep the PE saturated (HAM at 8/8):
  - batched DMAs: X in 4 gathers of 512 rows, each weight in one gather.
  - X^T via PE transposes (xps pool); weights used as f32r directly.
  - qT/kT [384,2048] bf16 projections (w^T x^T form); the softmax scale
    1/8 is folded into the qT copy so the ACT exp needs no scale.
  - v computed directly in [T, 384] orientation (lhsT = x^T chunk), no
    v transposes; ones column appended -> va[128, t, 6, 65] fp8 gives the
    softmax denominators out of the ctx matmul for free.
  - attention stream over (half, head, j): scores^T chunk (bf16) -> exp on
    ACT -> ctx^T accumulated with fp8 DoubleRow over j-PAIRS (the exp of
    j=2p,2p+1 lands in one [128,2048] fp8 tile = the DoubleRow rhs), ctx
    lagging scores by a pair so the PE never blocks on the ACT. All other
    projection work, X-transposes, and the output projection are queued as
    "extras" drained into the PE stream (forced prerequisite drains +
    token bucket), so the PE stays busy end-to-end.
  - normalize: R = ones x recip(sums) (PE outer product), ctxT = cu * R.
  - out[t] = sum_m ctxT[m].T @ Wo[m] -> DMA out (partial, pre-bias).

PSUM budget (8 banks): sps 2x[128,1024] (4) + cps [65,1024] (2) +
xps 2x[128,512] (2, shared by transposes/proj/v/rps/outproj).
"""
import sys
import os

sys.path.insert(0, "/opt/trn_rl_repo")

import numpy as np

P = 128
T = 2048
C = 768
HD = 384          # per-core head columns (6 heads x 64)
D = 64
NT = T // P       # 16 T chunks of 128
KC = C // P       # 6 contraction chunks for C
MC = HD // P      # 3 chunks of head dims
NH = 6            # heads per core
HALF = 1024       # T_q blocking for the attention inner loop
NN = T // 512     # 4 proj n-chunks of 512

_cache = {}


def _build():
    import concourse.bacc as bacc
    import concourse.mybir as mybir
    import concourse.tile as tile
    from concourse.masks import make_identity
    from contextlib import ExitStack

    F32 = mybir.dt.float32
    F32R = mybir.dt.float32r
    BF16 = mybir.dt.bfloat16
    FP8 = mybir.dt.float8e4
    AF = mybir.ActivationFunctionType
    ALU = mybir.AluOpType
    DR = mybir.MatmulPerfMode.DoubleRow

    nc = bacc.Bacc("TRN2", target_bir_lowering=False, debug=False)
    x = nc.dram_tensor("x", [T, C], BF16, kind="ExternalInput").ap()
    wq = nc.dram_tensor("wq", [C, HD], BF16, kind="ExternalInput").ap()
    wk = nc.dram_tensor("wk", [C, HD], BF16, kind="ExternalInput").ap()
    wv = nc.dram_tensor("wv", [C, HD], BF16, kind="ExternalInput").ap()
    wo = nc.dram_tensor("wo", [HD, C], F32R, kind="ExternalInput").ap()
    out = nc.dram_tensor("out", [T, C], F32, kind="ExternalOutput").ap()

    def emit(tc, ctx):
        consts = ctx.enter_context(tc.tile_pool(name="consts", bufs=1))
        ident = consts.tile([P, P], F32)
        make_identity(nc, ident)
        ident_bf = consts.tile([P, P], BF16)
        nc.vector.tensor_copy(ident_bf[:], ident[:])
        ones_r = consts.tile([1, D], F32R)
        nc.vector.tensor_scalar(ones_r[:], ident[0:1, 0:D], 0.0, 1.0, ALU.mult, ALU.add)

        big = ctx.enter_context(tc.tile_pool(name="big", bufs=1))
        xt = [big.tile([P, T], BF16, name=f"xt{kc}", tag=f"xt{kc}") for kc in range(KC)]
        qT = [big.tile([P, T], BF16, name=f"qT{m}", tag=f"qT{m}") for m in range(MC)]
        kT = [big.tile([P, T], BF16, name=f"kT{m}", tag=f"kT{m}") for m in range(MC)]
        ctxT = [big.tile([P, T], F32R, name=f"cT{m}", tag=f"cT{m}") for m in range(MC)]
        # va layout [p, h, t, 80]: cols 0..64 used (64 v dims + ones), rest pad
        VZ = 80
        va = big.tile([P, NH * NT * VZ], BF16, name="va", tag="va")
        va4 = va[:].rearrange("p (h t z) -> p h t z", h=NH, t=NT)
        # weight slabs: one DMA each; w[nm][:, 384*kc+...] is chunk kc
        wsl = {}
        for nm in ("q", "k", "v"):
            wsl[nm] = big.tile([P, KC * HD], BF16, name=f"w{nm}", tag=f"w{nm}")
        wosl = big.tile([P, MC * C], F32R, name="wo", tag="wo")
        w_r = {(nm, kc): wsl[nm][:, HD * kc:HD * (kc + 1)]
               for nm in ("q", "k", "v") for kc in range(KC)}
        wo_r = [wosl[:, C * m:C * (m + 1)] for m in range(MC)]

        xrp = ctx.enter_context(tc.tile_pool(name="xrp", bufs=2))
        ptp = ctx.enter_context(tc.tile_pool(name="ptp", bufs=3))
        outp = ctx.enter_context(tc.tile_pool(name="outp", bufs=2))
        norm = ctx.enter_context(tc.tile_pool(name="norm", bufs=1))
        xps = ctx.enter_context(tc.tile_pool(name="xps", bufs=2, space="PSUM"))
        sps = ctx.enter_context(tc.tile_pool(name="sps", bufs=2, space="PSUM"))
        cps = ctx.enter_context(tc.tile_pool(name="cps", bufs=1, space="PSUM"))

        # denominator ones column, before anything reads va
        nc.vector.tensor_scalar(
            va4[:, :, :, D:D + 1],
            ident[:, 0:NH * NT].rearrange("p (h t c) -> p h t c", h=NH, t=NT),
            0.0, 1.0, ALU.mult, ALU.add)

        # ---- batched input DMAs (X group g = rows [512g, 512(g+1)))
        xg = []
        def x_dma(g):
            xr = xrp.tile([P, 4 * C], BF16, name=f"xg{g}", tag="xr")
            nc.sync.dma_start(
                xr[:].rearrange("p (i c) -> p i c", i=4),
                x[512 * g:512 * (g + 1), :].rearrange("(i p) c -> p i c", p=P))
            xg.append(xr)
        x_dma(0)
        x_dma(1)
        for nm, src in (("k", wk), ("q", wq), ("v", wv)):
            nc.sync.dma_start(
                wsl[nm][:].rearrange("p (i c) -> p i c", i=KC),
                src[:].rearrange("(i p) c -> p i c", p=P))
        x_dma(2)
        x_dma(3)
        nc.sync.dma_start(
            wosl[:].rearrange("p (i c) -> p i c", i=MC),
            wo[:].rearrange("(i p) c -> p i c", p=P))

        # PE warm-up while the first DMAs land: dependency-free matmuls ramp
        # the HAM clock to 2.4 GHz before the real pipeline starts.
        for i in range(50):
            dps = xps.tile([P, 512], F32, name=f"warm{i}", tag="xps")
            nc.tensor.matmul(dps[:, 0:P], ident_bf[:], ident_bf[:],
                             start=True, stop=True)

        done = set()

        def xt_group(tq):
            """transpose X rows [512tq, 512(tq+1)) into xt[kc][:, 512tq:...]"""
            for kc in range(KC):
                # bf16 [128,512] = 1KB fits the 2KB "xps" slot; sharing the
                # tag keeps transposes off the sps ring (no exp-reader waits)
                tp = xps.tile([P, 512], BF16, name=f"tp{tq}_{kc}", tag="xps")
                for i in range(4):
                    nc.tensor.transpose(tp[:, P * i:P * (i + 1)],
                                        xg[tq][:, C * i + P * kc:C * i + P * (kc + 1)],
                                        ident_bf[:])
                nc.vector.tensor_copy(xt[kc][:, 512 * tq:512 * (tq + 1)], tp[:])
            done.add(("xt", tq))

        def qk_chunk(nm, m, n):
            """qT/kT[m][:, 512n:512(n+1)] (bf16); q gets the 1/8 scale"""
            dest = qT if nm == "q" else kT
            ps = xps.tile([P, 512], F32, name=f"ps{nm}{m}{n}", tag="xps")
            for kc in range(KC):
                nc.tensor.matmul(ps[:], w_r[nm, kc][:, P * m:P * (m + 1)],
                                 xt[kc][:, 512 * n:512 * (n + 1)],
                                 start=(kc == 0), stop=(kc == KC - 1))
            if nm == "q":
                nc.vector.tensor_scalar(dest[m][:, 512 * n:512 * (n + 1)], ps[:],
                                        float(D) ** -0.5, 0.0, ALU.mult, ALU.add)
            else:
                nc.vector.tensor_copy(dest[m][:, 512 * n:512 * (n + 1)], ps[:])
            done.add((f"qk{m}", nm, n))

        def v_chunk(t_i):
            """va[:, t_i, :, 0:64] = (X @ Wv) rows [128t_i, 128(t_i+1))  (fp8)"""
            ps = xps.tile([P, 512], F32, name=f"pv{t_i}", tag="xps")
            for kc in range(KC):
                nc.tensor.matmul(ps[:, 0:HD], xt[kc][:, P * t_i:P * (t_i + 1)],
                                 w_r["v", kc][:], start=(kc == 0), stop=(kc == KC - 1))
            nc.vector.tensor_copy(va4[:, :, t_i, 0:D],
                                  ps[:, 0:HD].rearrange("p (h c) -> p h c", h=NH))
            done.add(("v", t_i))

        def outproj(t_i):
            psA = xps.tile([P, 512], F32, name=f"psA{t_i}", tag="xps")
            psB = xps.tile([P, 512], F32, name=f"psB{t_i}", tag="xps")
            for m in range(MC):
                nc.tensor.matmul(psA[:], ctxT[m][:, P * t_i:P * (t_i + 1)],
                                 wo_r[m][:, 0:512], start=(m == 0), stop=(m == MC - 1))
                nc.tensor.matmul(psB[:, 0:C - 512], ctxT[m][:, P * t_i:P * (t_i + 1)],
                                 wo_r[m][:, 512:C], start=(m == 0), stop=(m == MC - 1))
            ob = outp.tile([P, C], F32, name=f"ob{t_i}", tag="ob")
            if t_i >= NT // 2:
                # tail chunks copy on the (idle) ACT engine so the DVE can
                # run the last normalize in parallel
                nc.scalar.copy(ob[:, 0:512], psA[:])
                nc.scalar.copy(ob[:, 512:C], psB[:, 0:C - 512])
            else:
                nc.vector.tensor_copy(ob[:, 0:512], psA[:])
                nc.vector.tensor_copy(ob[:, 512:C], psB[:, 0:C - 512])
            nc.sync.dma_start(out[P * t_i:P * (t_i + 1), :], ob[:])
            done.add(("po", t_i))

        # ---- extras queue: (label, cost_ns, thunk); queue order respects deps
        extras = []
        for t_i in range(2):
            extras.append((("v", t_i), 1300, lambda t_i=t_i: v_chunk(t_i)))
        extras.append((("qk0", "k", 1), 1400, lambda: qk_chunk("k", 0, 1)))
        extras.append((("xt", 2), 1100, lambda: xt_group(2)))
        extras.append((("xt", 3), 1100, lambda: xt_group(3)))
        for n in (2, 3):
            extras.append((("qk0", "k", n), 1400, lambda n=n: qk_chunk("k", 0, n)))
        for t_i in range(2, NT):
            extras.append((("v", t_i), 1300, lambda t_i=t_i: v_chunk(t_i)))
        for n in (2, 3):
            extras.append((("qk0", "q", n), 1400, lambda n=n: qk_chunk("q", 0, n)))
        for m in (1, 2):
            for nm, n in (("k", 0), ("k", 1), ("q", 0), ("q", 1), ("k", 2), ("k", 3)):
                extras.append(((f"qk{m}", nm, n), 1400,
                               lambda nm=nm, m=m, n=n: qk_chunk(nm, m, n)))
        for m in (1, 2):
            for n in (2, 3):
                extras.append(((f"qk{m}", "q", n), 1400,
                               lambda m=m, n=n: qk_chunk("q", m, n)))

        def drain_one():
            label, cost, th = extras.pop(0)
            th()
            return cost

        def drain_until(pred):
            while extras and not pred():
                drain_one()

        def need(*labels):
            req = set(labels)
            drain_until(lambda: req <= done)

        # ---- startup: enough for h0's first scores, emitted inline
        xt_group(0)
        qk_chunk("k", 0, 0)
        qk_chunk("q", 0, 0)
        xt_group(1)
        qk_chunk("q", 0, 1)

        # ---- the attention stream
        units = [(half, h, j) for half in (0, 1) for h in range(NH)
                 for j in range(NT)]
        cps_of = {}
        pt_of = {}
        pending = None     # (half, h, j) ctx waiting for emission
        bucket = 0.0

        def emit_scores_exp(u):
            half, h, j = u
            m, par = divmod(h, 2)
            need((f"qk{m}", "q", 2 * half), (f"qk{m}", "q", 2 * half + 1),
                 (f"qk{m}", "k", j // 4))
            qh = qT[m][D * par:D * (par + 1), HALF * half:HALF * (half + 1)]
            kh = kT[m][D * par:D * (par + 1), :]
            sp = sps.tile([P, HALF], F32, name=f"s{half}{h}{j}", tag="sps")
            for u2 in range(2):
                nc.tensor.matmul(sp[:, 512 * u2:512 * (u2 + 1)],
                                 kh[:, P * j:P * (j + 1)],
                                 qh[:, 512 * u2:512 * (u2 + 1)],
                                 start=True, stop=True)
            pt = ptp.tile([P, HALF], BF16, name=f"pt{half}{h}{j}", tag="pt")
            nc.scalar.activation(pt[:], sp[:], AF.Exp)
            pt_of[u] = pt

        def emit_ctx(pu):
            half, h, j = pu
            need(("v", j))
            if j == 0:
                cps_of[half, h] = cps.tile([D + 1, HALF], F32,
                                           name=f"c{half}{h}", tag="cps")
            cp = cps_of[half, h]
            pt = pt_of.pop(pu)
            for u2 in range(2):
                nc.tensor.matmul(cp[:, 512 * u2:512 * (u2 + 1)],
                                 va4[:, h, j, 0:D + 1],
                                 pt[:, 512 * u2:512 * (u2 + 1)],
                                 start=(j == 0), stop=(j == NT - 1))
            if j == NT - 1:
                normalize(half, h)

        def normalize(half, h):
            m, par = divmod(h, 2)
            cp = cps_of.pop((half, h))
            q0 = HALF * half
            # one evacuation copy frees the cps banks fast (next head's ctx
            # is gated on it); the sums row is then re-staged from SBUF to a
            # partition-0 tile (reciprocal_approx_fast needs offset-0 input)
            cu = norm.tile([D + 1, HALF], F32, name=f"cu{half}{h}", tag="cu")
            nc.vector.tensor_copy(cu[:], cp[0:D + 1, :])
            s_sb = norm.tile([1, HALF], F32, name=f"sb{half}{h}", tag="ssb")
            nc.vector.tensor_copy(s_sb[:], cu[D:D + 1, :])
            rr = norm.tile([1, HALF], F32, name=f"rr{half}{h}", tag="rr")
            nc.vector.reciprocal_approx_fast(rr[:], s_sb[:])
            rr_r = norm.tile([1, HALF], F32R, name=f"rc{half}{h}", tag="rrr")
            nc.vector.tensor_copy(rr_r[:], rr[:])
            for u2 in range(2):
                rp = xps.tile([P, 512], F32, name=f"rp{half}{h}{u2}", tag="xps")
                nc.tensor.matmul(rp[0:D, :], ones_r[:],
                                 rr_r[:][:, 512 * u2:512 * (u2 + 1)],
                                 start=True, stop=True)
                nc.vector.tensor_mul(
                    ctxT[m][D * par:D * (par + 1), q0 + 512 * u2:q0 + 512 * (u2 + 1)],
                    cu[0:D, 512 * u2:512 * (u2 + 1)], rp[0:D, :])
            if h == NH - 1:
                t_lo = 0 if half == 0 else NT // 2
                for t_i in range(t_lo, t_lo + NT // 2):
                    extras.append((("po", t_i), 1100, lambda t_i=t_i: outproj(t_i)))

        for u in units:
            emit_scores_exp(u)
            if pending is not None:
                emit_ctx(pending)
            pending = u
            bucket = min(bucket + 420.0, 4200.0)
            while extras and bucket >= extras[0][1]:
                bucket -= drain_one()
        emit_ctx(pending)
        while extras:
            drain_one()

    with tile.TileContext(nc) as tc, ExitStack() as ctx:
        emit(tc, ctx)

    nc.compile()
    return nc


def kernel(X, Wq, Wk, Wv, Wo, bo):
    from concourse import bass_utils

    if "nc" not in _cache:
        _cache["nc"] = _build()
    nc = _cache["nc"]

    from ml_dtypes import bfloat16

    X = np.asarray(X, dtype=np.float32).astype(bfloat16)
    Wq = np.asarray(Wq, np.float32).astype(bfloat16)
    Wk = np.asarray(Wk, np.float32).astype(bfloat16)
    Wv = np.asarray(Wv, np.float32).astype(bfloat16)
    in_maps = []
    for c in range(8):
        b, g = divmod(c, 2)
        sl = slice(HD * g, HD * (g + 1))
        in_maps.append({
            "x": np.ascontiguousarray(X[b]),
            "wq": np.ascontiguousarray(Wq[:, sl]),
            "wk": np.ascontiguousarray(Wk[:, sl]),
            "wv": np.ascontiguousarray(Wv[:, sl]),
            "wo": np.ascontiguousarray(np.asarray(Wo, np.float32)[sl, :]),
        })
    res = bass_utils.run_bass_kernel_spmd(nc, in_maps, core_ids=list(range(8)))
    _cache["last_res"] = res
    outf = np.empty((4, T, C), np.float32)
    bo = np.asarray(bo, np.float32)
    for b in range(4):
        outf[b] = res.results[2 * b]["out"] + res.results[2 * b + 1]["out"] + bo
    return outf


# revision 41
# speedup vs baseline: 1.1236x; 1.0243x over previous
"""Multi-head self-attention (B=4, T=2048, C=768, H=12) on 8 trn2 NeuronCores.

Sharding: core c -> batch b=c//2, head-group g=c%2 (6 heads each).
Each core computes its 6 heads' attention and a partial output projection
(contraction over its 384 ctx dims). Host sums the 2 partials per batch
and adds the bias.

Per-core kernel, software-pipelined to keep the PE saturated (HAM at 8/8):
  - batched DMAs: X in 4 gathers of 512 rows, each weight in one gather.
  - X^T via PE transposes (xps pool); weights used as f32r directly.
  - qT/kT [384,2048] bf16 projections (w^T x^T form); the softmax scale
    1/8 is folded into the qT copy so the ACT exp needs no scale.
  - v computed directly in [T, 384] orientation (lhsT = x^T chunk), no
    v transposes; ones column appended -> va[128, t, 6, 65] fp8 gives the
    softmax denominators out of the ctx matmul for free.
  - attention stream over (half, head, j): scores^T chunk (bf16) -> exp on
    ACT -> ctx^T accumulated with fp8 DoubleRow over j-PAIRS (the exp of
    j=2p,2p+1 lands in one [128,2048] fp8 tile = the DoubleRow rhs), ctx
    lagging scores by a pair so the PE never blocks on the ACT. All other
    projection work, X-transposes, and the output projection are queued as
    "extras" drained into the PE stream (forced prerequisite drains +
    token bucket), so the PE stays busy end-to-end.
  - normalize: R = ones x recip(sums) (PE outer product), ctxT = cu * R.
  - out[t] = sum_m ctxT[m].T @ Wo[m] -> DMA out (partial, pre-bias).

PSUM budget (8 banks): sps 2x[128,1024] (4) + cps [65,1024] (2) +
xps 2x[128,512] (2, shared by transposes/proj/v/rps/outproj).
"""
import sys
import os

sys.path.insert(0, "/opt/trn_rl_repo")

import numpy as np

P = 128
T = 2048
C = 768
HD = 384          # per-core head columns (6 heads x 64)
D = 64
NT = T // P       # 16 T chunks of 128
KC = C // P       # 6 contraction chunks for C
MC = HD // P      # 3 chunks of head dims
NH = 6            # heads per core
HALF = 1024       # T_q blocking for the attention inner loop
NN = T // 512     # 4 proj n-chunks of 512

_cache = {}


def _build():
    import concourse.bacc as bacc
    import concourse.mybir as mybir
    import concourse.tile as tile
    from concourse.masks import make_identity
    from contextlib import ExitStack

    F32 = mybir.dt.float32
    F32R = mybir.dt.float32r
    BF16 = mybir.dt.bfloat16
    FP8 = mybir.dt.float8e4
    AF = mybir.ActivationFunctionType
    ALU = mybir.AluOpType
    DR = mybir.MatmulPerfMode.DoubleRow

    nc = bacc.Bacc("TRN2", target_bir_lowering=False, debug=False)
    x = nc.dram_tensor("x", [T, C], BF16, kind="ExternalInput").ap()
    wq = nc.dram_tensor("wq", [C, HD], BF16, kind="ExternalInput").ap()
    wk = nc.dram_tensor("wk", [C, HD], BF16, kind="ExternalInput").ap()
    wv = nc.dram_tensor("wv", [C, HD], BF16, kind="ExternalInput").ap()
    wo = nc.dram_tensor("wo", [HD, C], BF16, kind="ExternalInput").ap()
    out = nc.dram_tensor("out", [T, C], F32, kind="ExternalOutput").ap()

    def emit(tc, ctx):
        consts = ctx.enter_context(tc.tile_pool(name="consts", bufs=1))
        ident = consts.tile([P, P], F32)
        make_identity(nc, ident)
        ident_bf = consts.tile([P, P], BF16)
        nc.vector.tensor_copy(ident_bf[:], ident[:])
        ones_r = consts.tile([1, D], F32R)
        nc.vector.tensor_scalar(ones_r[:], ident[0:1, 0:D], 0.0, 1.0, ALU.mult, ALU.add)

        big = ctx.enter_context(tc.tile_pool(name="big", bufs=1))
        xt = [big.tile([P, T], BF16, name=f"xt{kc}", tag=f"xt{kc}") for kc in range(KC)]
        qT = [big.tile([P, T], BF16, name=f"qT{m}", tag=f"qT{m}") for m in range(MC)]
        kT = [big.tile([P, T], BF16, name=f"kT{m}", tag=f"kT{m}") for m in range(MC)]
        ctxT = [big.tile([P, T], BF16, name=f"cT{m}", tag=f"cT{m}") for m in range(MC)]
        # va layout [p, h, t, 80]: cols 0..64 used (64 v dims + ones), rest pad
        VZ = 80
        va = big.tile([P, NH * NT * VZ], BF16, name="va", tag="va")
        va4 = va[:].rearrange("p (h t z) -> p h t z", h=NH, t=NT)
        # weight slabs: one DMA each; w[nm][:, 384*kc+...] is chunk kc
        wsl = {}
        for nm in ("q", "k", "v"):
            wsl[nm] = big.tile([P, KC * HD], BF16, name=f"w{nm}", tag=f"w{nm}")
        wosl = big.tile([P, MC * C], BF16, name="wo", tag="wo")
        w_r = {(nm, kc): wsl[nm][:, HD * kc:HD * (kc + 1)]
               for nm in ("q", "k", "v") for kc in range(KC)}
        wo_r = [wosl[:, C * m:C * (m + 1)] for m in range(MC)]

        xrp = ctx.enter_context(tc.tile_pool(name="xrp", bufs=2))
        ptp = ctx.enter_context(tc.tile_pool(name="ptp", bufs=3))
        outp = ctx.enter_context(tc.tile_pool(name="outp", bufs=2))
        norm = ctx.enter_context(tc.tile_pool(name="norm", bufs=1))
        xps = ctx.enter_context(tc.tile_pool(name="xps", bufs=2, space="PSUM"))
        sps = ctx.enter_context(tc.tile_pool(name="sps", bufs=2, space="PSUM"))
        cps = ctx.enter_context(tc.tile_pool(name="cps", bufs=1, space="PSUM"))

        # denominator ones column, before anything reads va
        nc.vector.tensor_scalar(
            va4[:, :, :, D:D + 1],
            ident[:, 0:NH * NT].rearrange("p (h t c) -> p h t c", h=NH, t=NT),
            0.0, 1.0, ALU.mult, ALU.add)

        # ---- batched input DMAs (X group g = rows [512g, 512(g+1)))
        xg = []
        def x_dma(g):
            xr = xrp.tile([P, 4 * C], BF16, name=f"xg{g}", tag="xr")
            nc.sync.dma_start(
                xr[:].rearrange("p (i c) -> p i c", i=4),
                x[512 * g:512 * (g + 1), :].rearrange("(i p) c -> p i c", p=P))
            xg.append(xr)
        x_dma(0)
        x_dma(1)
        for nm, src in (("k", wk), ("q", wq), ("v", wv)):
            nc.sync.dma_start(
                wsl[nm][:].rearrange("p (i c) -> p i c", i=KC),
                src[:].rearrange("(i p) c -> p i c", p=P))
        x_dma(2)
        x_dma(3)
        nc.sync.dma_start(
            wosl[:].rearrange("p (i c) -> p i c", i=MC),
            wo[:].rearrange("(i p) c -> p i c", p=P))

        # PE warm-up while the first DMAs land: dependency-free matmuls ramp
        # the HAM clock to 2.4 GHz before the real pipeline starts.
        for i in range(36):
            dps = xps.tile([P, 512], F32, name=f"warm{i}", tag="xps")
            nc.tensor.matmul(dps[:, 0:P], ident_bf[:], ident_bf[:],
                             start=True, stop=True)

        done = set()

        def xt_group(tq):
            """transpose X rows [512tq, 512(tq+1)) into xt[kc][:, 512tq:...]"""
            for kc in range(KC):
                # bf16 [128,512] = 1KB fits the 2KB "xps" slot; sharing the
                # tag keeps transposes off the sps ring (no exp-reader waits)
                tp = xps.tile([P, 512], BF16, name=f"tp{tq}_{kc}", tag="xps")
                for i in range(4):
                    nc.tensor.transpose(tp[:, P * i:P * (i + 1)],
                                        xg[tq][:, C * i + P * kc:C * i + P * (kc + 1)],
                                        ident_bf[:])
                nc.vector.tensor_copy(xt[kc][:, 512 * tq:512 * (tq + 1)], tp[:])
            done.add(("xt", tq))

        def qk_chunk(nm, m, n):
            """qT/kT[m][:, 512n:512(n+1)] (bf16); q gets the 1/8 scale"""
            dest = qT if nm == "q" else kT
            ps = xps.tile([P, 512], F32, name=f"ps{nm}{m}{n}", tag="xps")
            for kc in range(KC):
                nc.tensor.matmul(ps[:], w_r[nm, kc][:, P * m:P * (m + 1)],
                                 xt[kc][:, 512 * n:512 * (n + 1)],
                                 start=(kc == 0), stop=(kc == KC - 1))
            if nm == "q":
                nc.vector.tensor_scalar(dest[m][:, 512 * n:512 * (n + 1)], ps[:],
                                        float(D) ** -0.5, 0.0, ALU.mult, ALU.add)
            else:
                nc.vector.tensor_copy(dest[m][:, 512 * n:512 * (n + 1)], ps[:])
            done.add((f"qk{m}", nm, n))

        def v_chunk(t_i):
            """va[:, t_i, :, 0:64] = (X @ Wv) rows [128t_i, 128(t_i+1))  (fp8)"""
            ps = xps.tile([P, 512], F32, name=f"pv{t_i}", tag="xps")
            for kc in range(KC):
                nc.tensor.matmul(ps[:, 0:HD], xt[kc][:, P * t_i:P * (t_i + 1)],
                                 w_r["v", kc][:], start=(kc == 0), stop=(kc == KC - 1))
            nc.vector.tensor_copy(va4[:, :, t_i, 0:D],
                                  ps[:, 0:HD].rearrange("p (h c) -> p h c", h=NH))
            done.add(("v", t_i))

        def outproj(t_i):
            psA = xps.tile([P, 512], F32, name=f"psA{t_i}", tag="xps")
            psB = xps.tile([P, 512], F32, name=f"psB{t_i}", tag="xps")
            for m in range(MC):
                nc.tensor.matmul(psA[:], ctxT[m][:, P * t_i:P * (t_i + 1)],
                                 wo_r[m][:, 0:512], start=(m == 0), stop=(m == MC - 1))
                nc.tensor.matmul(psB[:, 0:C - 512], ctxT[m][:, P * t_i:P * (t_i + 1)],
                                 wo_r[m][:, 512:C], start=(m == 0), stop=(m == MC - 1))
            ob = outp.tile([P, C], F32, name=f"ob{t_i}", tag="ob")
            if t_i >= NT // 2:
                # tail chunks copy on the (idle) ACT engine so the DVE can
                # run the last normalize in parallel
                nc.scalar.copy(ob[:, 0:512], psA[:])
                nc.scalar.copy(ob[:, 512:C], psB[:, 0:C - 512])
            else:
                nc.vector.tensor_copy(ob[:, 0:512], psA[:])
                nc.vector.tensor_copy(ob[:, 512:C], psB[:, 0:C - 512])
            nc.sync.dma_start(out[P * t_i:P * (t_i + 1), :], ob[:])
            done.add(("po", t_i))

        # ---- extras queue: (label, cost_ns, thunk); queue order respects deps
        extras = []
        for t_i in range(2):
            extras.append((("v", t_i), 1300, lambda t_i=t_i: v_chunk(t_i)))
        extras.append((("qk0", "k", 1), 1400, lambda: qk_chunk("k", 0, 1)))
        extras.append((("xt", 2), 1100, lambda: xt_group(2)))
        extras.append((("xt", 3), 1100, lambda: xt_group(3)))
        for n in (2, 3):
            extras.append((("qk0", "k", n), 1400, lambda n=n: qk_chunk("k", 0, n)))
        for t_i in range(2, NT):
            extras.append((("v", t_i), 1300, lambda t_i=t_i: v_chunk(t_i)))
        for n in (2, 3):
            extras.append((("qk0", "q", n), 1400, lambda n=n: qk_chunk("q", 0, n)))
        for m in (1, 2):
            for nm, n in (("k", 0), ("k", 1), ("q", 0), ("q", 1), ("k", 2), ("k", 3)):
                extras.append(((f"qk{m}", nm, n), 1400,
                               lambda nm=nm, m=m, n=n: qk_chunk(nm, m, n)))
        for m in (1, 2):
            for n in (2, 3):
                extras.append(((f"qk{m}", "q", n), 1400,
                               lambda m=m, n=n: qk_chunk("q", m, n)))

        def drain_one():
            label, cost, th = extras.pop(0)
            th()
            return cost

        def drain_until(pred):
            while extras and not pred():
                drain_one()

        def need(*labels):
            req = set(labels)
            drain_until(lambda: req <= done)

        # ---- startup: enough for h0's first scores, emitted inline
        xt_group(0)
        qk_chunk("k", 0, 0)
        qk_chunk("q", 0, 0)
        xt_group(1)
        qk_chunk("q", 0, 1)

        # ---- the attention stream
        units = [(half, h, j) for half in (0, 1) for h in range(NH)
                 for j in range(NT)]
        cps_of = {}
        pt_of = {}
        pending = None     # (half, h, j) ctx waiting for emission
        bucket = 0.0

        def emit_scores_exp(u):
            half, h, j = u
            m, par = divmod(h, 2)
            need((f"qk{m}", "q", 2 * half), (f"qk{m}", "q", 2 * half + 1),
                 (f"qk{m}", "k", j // 4))
            qh = qT[m][D * par:D * (par + 1), HALF * half:HALF * (half + 1)]
            kh = kT[m][D * par:D * (par + 1), :]
            sp = sps.tile([P, HALF], F32, name=f"s{half}{h}{j}", tag="sps")
            for u2 in range(2):
                nc.tensor.matmul(sp[:, 512 * u2:512 * (u2 + 1)],
                                 kh[:, P * j:P * (j + 1)],
                                 qh[:, 512 * u2:512 * (u2 + 1)],
                                 start=True, stop=True)
            pt = ptp.tile([P, HALF], BF16, name=f"pt{half}{h}{j}", tag="pt")
            nc.scalar.activation(pt[:], sp[:], AF.Exp)
            pt_of[u] = pt

        def emit_ctx(pu):
            half, h, j = pu
            need(("v", j))
            if j == 0:
                cps_of[half, h] = cps.tile([D + 1, HALF], F32,
                                           name=f"c{half}{h}", tag="cps")
            cp = cps_of[half, h]
            pt = pt_of.pop(pu)
            for u2 in range(2):
                nc.tensor.matmul(cp[:, 512 * u2:512 * (u2 + 1)],
                                 va4[:, h, j, 0:D + 1],
                                 pt[:, 512 * u2:512 * (u2 + 1)],
                                 start=(j == 0), stop=(j == NT - 1))
            if j == NT - 1:
                normalize(half, h)

        def normalize(half, h):
            m, par = divmod(h, 2)
            cp = cps_of.pop((half, h))
            q0 = HALF * half
            # one evacuation copy frees the cps banks fast (next head's
            # ctx is gated on it); the sums row is then re-staged to a
            # partition-0 tile (reciprocal_approx_fast needs offset-0 in)
            cu = norm.tile([D + 1, HALF], F32, name=f"cu{half}{h}", tag="cu")
            nc.vector.tensor_copy(cu[:], cp[0:D + 1, :])
            s_sb = norm.tile([1, HALF], F32, name=f"sb{half}{h}", tag="ssb")
            nc.vector.tensor_copy(s_sb[:], cu[D:D + 1, :])
            rr = norm.tile([1, HALF], F32, name=f"rr{half}{h}", tag="rr")
            nc.vector.reciprocal_approx_fast(rr[:], s_sb[:])
            rr_r = norm.tile([1, HALF], F32R, name=f"rc{half}{h}", tag="rrr")
            nc.vector.tensor_copy(rr_r[:], rr[:])
            for u2 in range(2):
                rp = xps.tile([P, 512], F32, name=f"rp{half}{h}{u2}", tag="xps")
                nc.tensor.matmul(rp[0:D, :], ones_r[:],
                                 rr_r[:][:, 512 * u2:512 * (u2 + 1)],
                                 start=True, stop=True)
                nc.vector.tensor_mul(
                    ctxT[m][D * par:D * (par + 1), q0 + 512 * u2:q0 + 512 * (u2 + 1)],
                    cu[0:D, 512 * u2:512 * (u2 + 1)], rp[0:D, :])
            if h == NH - 1:
                t_lo = 0 if half == 0 else NT // 2
                for t_i in range(t_lo, t_lo + NT // 2):
                    extras.append((("po", t_i), 1100, lambda t_i=t_i: outproj(t_i)))

        for u in units:
            emit_scores_exp(u)
            if pending is not None:
                emit_ctx(pending)
            pending = u
            bucket = min(bucket + 420.0, 4200.0)
            while extras and bucket >= extras[0][1]:
                bucket -= drain_one()
        emit_ctx(pending)
        while extras:
            drain_one()

    with tile.TileContext(nc) as tc, ExitStack() as ctx:
        emit(tc, ctx)

    nc.compile()
    return nc


def kernel(X, Wq, Wk, Wv, Wo, bo):
    from concourse import bass_utils

    if "nc" not in _cache:
        _cache["nc"] = _build()
    nc = _cache["nc"]

    from ml_dtypes import bfloat16

    X = np.asarray(X, dtype=np.float32).astype(bfloat16)
    Wq = np.asarray(Wq, np.float32).astype(bfloat16)
    Wk = np.asarray(Wk, np.float32).astype(bfloat16)
    Wv = np.asarray(Wv, np.float32).astype(bfloat16)
    Wo = np.asarray(Wo, np.float32).astype(bfloat16)
    in_maps = []
    for c in range(8):
        b, g = divmod(c, 2)
        sl = slice(HD * g, HD * (g + 1))
        in_maps.append({
            "x": np.ascontiguousarray(X[b]),
            "wq": np.ascontiguousarray(Wq[:, sl]),
            "wk": np.ascontiguousarray(Wk[:, sl]),
            "wv": np.ascontiguousarray(Wv[:, sl]),
            "wo": np.ascontiguousarray(Wo[sl, :]),
        })
    res = bass_utils.run_bass_kernel_spmd(nc, in_maps, core_ids=list(range(8)))
    _cache["last_res"] = res
    outf = np.empty((4, T, C), np.float32)
    bo = np.asarray(bo, np.float32)
    for b in range(4):
        outf[b] = res.results[2 * b]["out"] + res.results[2 * b + 1]["out"] + bo
    return outf
